# revision 1
# baseline (speedup 1.0000x reference)
"""Single-head causal self-attention on 8 trn2 NeuronCores.

Problem: x[4,4096,1024], Wq/Wk/Wv[1024,128]+biases -> causal attention out
[4,4096,128], fp32.

Sharding: core c = (b = c//2, j = c%2). Core (b, j) handles batch b and the
K/V column 128-blocks of parity j (alternating blocks balance the causal
triangle). It computes, for ALL 4096 query rows, the *unnormalized* partial
attention over its own columns:
    O_un^T[h, s] = sum_{t in cols_j, t<=s} exp(q_s.k_t * scale) * v_t[h]
    l[s]         = sum_{t in cols_j, t<=s} exp(q_s.k_t * scale)
Host combines:  O[s] = (O_un0[s] + O_un1[s]) / (l0[s] + l1[s]).
No per-core max subtraction is needed: scores are ~N(0,1) (bounded ~6), so
exp never overflows; masked entries get an additive -1e32 -> exp = 0.

SPMD uniformity: the same Bass program runs on all 8 cores. Parity enters
only through data: for j=1 the host swaps adjacent 128-row blocks of x
(involution), so "even position blocks" on the device are the core's own
columns; the causal masks (2 tiles of [128,512], R-independent by algebra)
are passed as per-core inputs. Output comes back in position space and the
host un-swaps.

The host passes x^T (x transposed, [1024, 4096]) declared float32r, so the
device needs no transposes for the projections and the DMA layout stays
line-rate (per-partition contiguous rows).

Device pipeline per core (all matmuls float32r, ~1.5e-4 component rel err,
~3.5e-4 end-to-end vs fp32 reference):
  Stage A (per 512-row superstep i2): DMA x^T tile [128, 8, 512] ->
    projections with W stationary: Q^T[h,s] (all s), K^T[h,t], V^T[h,t]
    (even position blocks only) -> ACT copy+bias; V^T PE-transposed to
    V[t,h]. Interleaved with stage B groups as their inputs complete.
  Stage B (per superblock R of 512 rows, pairs p=0..R of t-blocks):
    S^T[t,s] = K^T.T @ Q^T  (PSUM, 2 banks per pair)
    last pair: DVE adds mask; ACT: P^T = exp(scale * S^T) -> f32r SBUF
    O^T += V.T @ P^T ; l += ones.T @ P^T  (PSUM accum over all 2R+2 blocks)
    copy to SBUF, DMA O^T[128,512] and l[1,512] -> DRAM.
"""

import sys

sys.path.insert(0, "/opt/trn_rl_repo")

import numpy as np

import concourse.bacc as bacc
import concourse.mybir as mybir
import concourse.tile as tile
from concourse import bass_utils
from concourse.masks import make_identity

S, E, H, B = 4096, 1024, 128, 4
NSUP, SUP = 8, 512
NEG = -1.0e32
SCALE = 1.0 / float(np.sqrt(128.0))
F32 = mybir.dt.float32
F32R = mybir.dt.float32r
ACT_IDENT = mybir.ActivationFunctionType.Identity
ACT_EXP = mybir.ActivationFunctionType.Exp


def build_nc(loop_n=None, bias_engine="act", pt_bufs=4, stage_b=True):
    nc = bacc.Bacc("TRN2", debug=False, num_devices=8)
    xt_d = nc.dram_tensor("xt", [E, S], F32R, kind="ExternalInput").ap()
    wq_d = nc.dram_tensor("wq", [E, H], F32R, kind="ExternalInput").ap()
    wk_d = nc.dram_tensor("wk", [E, H], F32R, kind="ExternalInput").ap()
    wv_d = nc.dram_tensor("wv", [E, H], F32R, kind="ExternalInput").ap()
    bias_d = nc.dram_tensor("bias", [H, 3], F32, kind="ExternalInput").ap()
    mask_d = nc.dram_tensor("mask", [128, 2, SUP], F32, kind="ExternalInput").ap()
    outT_d = nc.dram_tensor("outT", [H, S], F32, kind="ExternalOutput").ap()
    l_d = nc.dram_tensor("lsum", [1, S], F32, kind="ExternalOutput").ap()

    with tile.TileContext(nc) as tc:
        with (
            tc.tile_pool(name="persist", bufs=1) as pp,
            tc.tile_pool(name="xts", bufs=3) as xtp,
            tc.tile_pool(name="vtmp", bufs=2) as vtp,
            tc.tile_pool(name="pts", bufs=pt_bufs) as ptp,
            tc.tile_pool(name="osb", bufs=2) as osp,
            tc.tile_pool(name="psproj", bufs=2, space="PSUM") as prp,
            tc.tile_pool(name="psst", bufs=2, space="PSUM") as stp,
            tc.tile_pool(name="pso", bufs=1, space="PSUM") as outp,
            tc.tile_pool(name="psl", bufs=1, space="PSUM") as lp,
        ):
            xt_view = xt_d.rearrange("(c p) s -> p c s", p=128)
            xT_tiles = {}

            def dma_xT(i2, pieces=2):
                xT = xtp.tile([128, 8, SUP], F32R, tag="xT", name="xT")
                w = 8 // pieces
                for hh in range(pieces):
                    nc.sync.dma_start(
                        xT[:, w * hh : w * hh + w, :],
                        xt_view[:, w * hh : w * hh + w, i2 * SUP : (i2 + 1) * SUP],
                    )
                xT_tiles[i2] = xT

            # DMA order tuned for PE start latency: wq + bias first, then the
            # first x^T half (enough for Q-proj chunks 0-3), then the rest.
            w_r = {}
            wq_s = pp.tile([128, 8, H], F32R, name="wr_q")
            nc.sync.dma_start(wq_s, wq_d.rearrange("(c p) h -> p c h", p=128))
            w_r["q"] = wq_s
            bias_sb = pp.tile([128, 3], F32)
            nc.sync.dma_start(bias_sb, bias_d)
            bias_s = {"q": bias_sb[:, 0:1], "k": bias_sb[:, 1:2], "v": bias_sb[:, 2:3]}
            dma_xT(0, pieces=4)
            for nm, wd in (("k", wk_d), ("v", wv_d)):
                wr = pp.tile([128, 8, H], F32R, name=f"wr_{nm}")
                nc.sync.dma_start(wr, wd.rearrange("(c p) h -> p c h", p=128))
                w_r[nm] = wr
            dma_xT(1)

            ident = pp.tile([128, 128], F32)
            make_identity(nc, ident)
            ones_f32 = pp.tile([128, 1], F32)
            nc.vector.memset(ones_f32, 1.0)
            ones_col = pp.tile([128, 1], F32R)
            nc.vector.tensor_copy(ones_col, ones_f32)

            mask_s = pp.tile([128, 2, SUP], F32)
            nc.sync.dma_start(mask_s, mask_d)

            qt_all = pp.tile([128, S], F32R)
            kt_all = pp.tile([128, 16, 128], F32R)
            v_all = pp.tile([128, 16, 128], F32R)

            o_ps = {}
            l_ps = {}
            pt_of = {}
            # per-R pair processing order (natural; flags mark first/last)
            order_of = {}
            for _R in range(NSUP):
                for _p in range(_R + 1):
                    order_of[(_R, _p)] = (_p == 0, _p == _R)

            def bias_copy(out_ap, in_ap, bias_ap):
                if bias_engine == "act":
                    nc.scalar.activation(out_ap, in_ap, ACT_IDENT, bias=bias_ap)
                else:
                    nc.vector.tensor_scalar_add(out_ap, in_ap, bias_ap)

            def emit_A(i2):
                if i2 + 2 < NSUP:
                    dma_xT(i2 + 2)
                xT = xT_tiles.pop(i2)
                xT4 = xT.rearrange("p c (t w) -> p c t w", t=4)

                qt_ps = prp.tile([128, SUP], F32, tag="proj", name="qt_ps")
                for c in range(8):
                    nc.tensor.matmul(
                        qt_ps,
                        w_r["q"][:, c, :],
                        xT[:, c, :],
                        start=(c == 0),
                        stop=(c == 7),
                    )
                bias_copy(qt_all[:, i2 * SUP : (i2 + 1) * SUP], qt_ps, bias_s["q"])

                kt_ps = prp.tile([128, 256], F32, tag="proj", name="kt_ps")
                for c in range(8):
                    nc.tensor.matmul(
                        kt_ps,
                        w_r["k"][:, c, :],
                        xT4[:, c, 0::2, :],
                        start=(c == 0),
                        stop=(c == 7),
                    )
                bias_copy(
                    kt_all[:, 2 * i2 : 2 * i2 + 2, :],
                    kt_ps.rearrange("p (t w) -> p t w", t=2),
                    bias_s["k"],
                )

                vt_ps = prp.tile([128, 256], F32, tag="proj", name="vt_ps")
                for c in range(8):
                    nc.tensor.matmul(
                        vt_ps,
                        w_r["v"][:, c, :],
                        xT4[:, c, 0::2, :],
                        start=(c == 0),
                        stop=(c == 7),
                    )
                vt_tmp = vtp.tile([128, 256], F32, tag="vtmp", name="vt_tmp")
                bias_copy(vt_tmp, vt_ps, bias_s["v"])
                v_ps2 = prp.tile([128, 256], F32, tag="proj", name="v_ps2")
                for tt in range(2):
                    nc.tensor.transpose(
                        v_ps2[:, tt * 128 : (tt + 1) * 128],
                        vt_tmp[:, tt * 128 : (tt + 1) * 128],
                        ident,
                    )
                nc.vector.tensor_copy(
                    v_all[:, 2 * i2 : 2 * i2 + 2, :],
                    v_ps2.rearrange("p (t w) -> p t w", t=2),
                )

            def emit_S(task):
                R, p = task
                st = stp.tile([128, 2, SUP], F32, tag="st", name="st")
                for half in range(2):
                    k = 2 * p + half
                    nc.tensor.matmul(
                        st[:, half, :],
                        kt_all[:, k, :],
                        qt_all[:, R * SUP : (R + 1) * SUP],
                        start=True,
                        stop=True,
                    )
                pt = ptp.tile([128, 2, SUP], F32R, tag="pt", name="pt")
                nc.scalar.activation(pt, st, ACT_EXP, scale=SCALE)
                if p == R:
                    nc.vector.tensor_mul(pt, pt, mask_s)
                pt_of[task] = pt

            def emit_AV(task):
                R, p = task
                first_p, last_p = order_of[(R, p)]
                if first_p:
                    o_ps[R] = outp.tile([128, SUP], F32, tag="o", name="o_ps")
                    l_ps[R] = lp.tile([128, SUP], F32, tag="l", name="l_ps")
                pt = pt_of.pop(task)
                for half in range(2):
                    k = 2 * p + half
                    nc.tensor.matmul(
                        o_ps[R],
                        v_all[:, k, :],
                        pt[:, half, :],
                        start=(first_p and half == 0),
                        stop=(last_p and half == 1),
                    )
                    nc.tensor.matmul(
                        l_ps[R][0:1, :],
                        ones_col,
                        pt[:, half, :],
                        start=(first_p and half == 0),
                        stop=(last_p and half == 1),
                    )
                if last_p:
                    o_sb = osp.tile([128, SUP], F32, tag="o_sb", name="o_sb")
                    nc.vector.tensor_copy(o_sb, o_ps[R])
                    nc.sync.dma_start(outT_d[:, R * SUP : (R + 1) * SUP], o_sb)
                    l_sb = osp.tile([1, SUP], F32, tag="l_sb", name="l_sb")
                    nc.vector.tensor_copy(l_sb, l_ps[R][0:1, :])
                    nc.sync.dma_start(l_d[:, R * SUP : (R + 1) * SUP], l_sb)

            pipe = {"prev": None}

            def push_task(task):
                emit_S(task)
                if pipe["prev"] is not None:
                    emit_AV(pipe["prev"])
                pipe["prev"] = task

            def emit_body():
                pipe["prev"] = None
                for i2 in range(NSUP):
                    emit_A(i2)
                    # superblock R=i2 is fully ready after A(i2): its qt slice
                    # is written there and its t-blocks k<=2*i2+1 live in
                    # supersteps k//2 <= i2
                    if stage_b:
                        for p in range(i2 + 1):
                            push_task((i2, p))
                if stage_b:
                    emit_AV(pipe["prev"])

            if loop_n is None:
                emit_body()
            else:
                with tc.For_i(0, loop_n, 1):
                    emit_body()

    nc.compile()
    return nc


def _perm1():
    idx = np.arange(S)
    return (idx // 128 ^ 1) * 128 + idx % 128


def _mask_for(j):
    ti = np.arange(128)[:, None, None]
    m = np.arange(2)[None, :, None]
    si = np.arange(SUP)[None, None, :]
    orig_s = 128 * ((si // 128) ^ j) + si % 128
    vis = orig_s >= 128 * (2 * m + j) + ti
    return np.where(vis, np.float32(1.0), np.float32(0.0)).astype(np.float32)


_CACHE = {}


def kernel(x, Wq, bq, Wk, bk, Wv, bv):
    if "nc" not in _CACHE:
        _CACHE["nc"] = build_nc()
    nc = _CACHE["nc"]

    x = np.ascontiguousarray(np.asarray(x, dtype=np.float32))
    Wq = np.ascontiguousarray(np.asarray(Wq, dtype=np.float32))
    Wk = np.ascontiguousarray(np.asarray(Wk, dtype=np.float32))
    Wv = np.ascontiguousarray(np.asarray(Wv, dtype=np.float32))
    bq = np.ascontiguousarray(np.asarray(bq, dtype=np.float32))
    bk = np.ascontiguousarray(np.asarray(bk, dtype=np.float32))
    bv = np.ascontiguousarray(np.asarray(bv, dtype=np.float32))

    perm = _perm1()
    masks = {j: _mask_for(j) for j in (0, 1)}
    # x^T per batch, and the column-block-swapped variant for parity-1 cores
    xT = {}
    for b in range(B):
        t = np.ascontiguousarray(x[b].T)  # [E, S]
        xT[(b, 0)] = t
        xT[(b, 1)] = np.ascontiguousarray(
            t.reshape(E, S // 128, 128)[:, (np.arange(S // 128) ^ 1), :].reshape(E, S)
        )

    in_maps = []
    for c in range(8):
        b, j = divmod(c, 2)
        in_maps.append(
            {
                "xt": xT[(b, j)],
                "wq": Wq,
                "wk": Wk,
                "wv": Wv,
                "bias": np.ascontiguousarray(np.stack([bq, bk, bv], axis=1)),
                "mask": masks[j],
            }
        )

    res = bass_utils.run_bass_kernel_spmd(nc, in_maps, core_ids=list(range(8)))

    out = np.empty((B, S, H), np.float32)
    for b in range(B):
        oT0 = res.results[2 * b]["outT"]
        l0 = res.results[2 * b]["lsum"][0]
        oT1 = res.results[2 * b + 1]["outT"][:, perm]
        l1 = res.results[2 * b + 1]["lsum"][0][perm]
        out[b] = ((oT0 + oT1) / (l0 + l1)[None, :]).T
    return out



# revision 5
# speedup vs baseline: 1.1704x; 1.1704x over previous
"""Single-head causal self-attention on 8 trn2 NeuronCores.

Problem: x[4,4096,1024], Wq/Wk/Wv[1024,128]+biases -> causal attention out
[4,4096,128], fp32.

Sharding: core c = (b = c//2, j = c%2). Core (b, j) handles batch b and the
K/V column 128-blocks of parity j (alternating blocks balance the causal
triangle). It computes, for ALL 4096 query rows, the *unnormalized* partial
attention over its own columns:
    O_un^T[h, s] = sum_{t in cols_j, t<=s} exp(q_s.k_t * scale) * v_t[h]
    l[s]         = sum_{t in cols_j, t<=s} exp(q_s.k_t * scale)
Host combines:  O[s] = (O_un0[s] + O_un1[s]) / (l0[s] + l1[s]) + bv.
No per-core max subtraction is needed: scores are ~N(0,1) (bounded ~6), so
exp never overflows; masked entries get multiplied by a 0/1 mask after exp.

Bias algebra: softmax((q+bq).(k+bk)) == softmax((q+bq).k) because the
(q+bq).bk term is constant in t and cancels in the combined normalization
(both cores use the same biased-Q/unbiased-K convention). The V bias moves
to the host: O = sum w (v+bv) / sum w = sum w v / sum w + bv. So the device
applies only the Q bias.

SPMD uniformity: the same Bass program runs on all 8 cores. Parity enters
only through data: for j=1 the host swaps adjacent 128-row blocks of x
(involution), so "even position blocks" on the device are the core's own
columns; the causal masks (2 tiles of [128,512], R-independent by algebra)
are passed as per-core inputs. Output comes back in position space and the
host un-swaps.

Precision: all matmul operands are bf16 (x^T, W, Q^T, K^T, V, P); PSUM
accumulation is f32; exp input (scores) is f32. Measured end-to-end rel
err ~4e-3 vs the f32 reference (tolerance 2e-2).

Device pipeline per core (per 512-row superstep i2):
  Stage A: DMA x^T tile [128, 8, 512] bf16 -> projections with W stationary:
    Q^T (all s, ACT copy+bias -> bf16), K^T / V^T (even position blocks only,
    DVE copy -> bf16); V^T PE-transposed to V[t,h] (bf16 identity).
  Stage B (superblock R = i2, pairs of own t-blocks, DIAGONAL pair first):
    S^T[t,s] = K^T.T @ Q^T  (PSUM f32, 2 banks per pair)
    ACT: exp(scale * S^T) -> bf16. Diagonal pair writes straight into the
    l-accumulator tile and gets the 0/1 mask (DVE mul); later pairs write a
    pt tile and DVE-add into the accumulator.
    O^T += V.T @ P^T (PSUM accum over all 2R+2 blocks)
    After the last pair: l[1,512] = ones.T @ acc (2 matmuls, PSUM-accumulated
    over the two halves) -> copies -> DMA O^T[128,512] f32 and l[1,512] f32.
"""

import sys

sys.path.insert(0, "/opt/trn_rl_repo")

import numpy as np

import concourse.bacc as bacc
import concourse.mybir as mybir
import concourse.tile as tile
from concourse import bass_utils
from concourse.masks import make_identity

S, E, H, B = 4096, 1024, 128, 4
NSUP, SUP = 8, 512
SCALE = 1.0 / float(np.sqrt(128.0))
F32 = mybir.dt.float32
BF16 = mybir.dt.bfloat16
ACT_IDENT = mybir.ActivationFunctionType.Identity
ACT_EXP = mybir.ActivationFunctionType.Exp

try:
    import ml_dtypes

    NP_BF16 = ml_dtypes.bfloat16
except ImportError:  # pragma: no cover
    NP_BF16 = None


def build_nc(loop_n=None):
    nc = bacc.Bacc("TRN2", debug=False, num_devices=8)
    xt_d = nc.dram_tensor("xt", [E, S], BF16, kind="ExternalInput").ap()
    wq_d = nc.dram_tensor("wq", [E, H], BF16, kind="ExternalInput").ap()
    wk_d = nc.dram_tensor("wk", [E, H], BF16, kind="ExternalInput").ap()
    wv_d = nc.dram_tensor("wv", [E, H], BF16, kind="ExternalInput").ap()
    bias_d = nc.dram_tensor("bias", [H, 1], F32, kind="ExternalInput").ap()
    mask_d = nc.dram_tensor("mask", [128, 2, SUP], BF16, kind="ExternalInput").ap()
    outT_d = nc.dram_tensor("outT", [H, S], F32, kind="ExternalOutput").ap()
    l_d = nc.dram_tensor("lsum", [1, S], F32, kind="ExternalOutput").ap()

    with tile.TileContext(nc) as tc:
        with (
            tc.tile_pool(name="persist", bufs=1) as pp,
            tc.tile_pool(name="xts", bufs=3) as xtp,
            tc.tile_pool(name="vtmp", bufs=2) as vtp,
            tc.tile_pool(name="pts", bufs=4) as ptp,
            tc.tile_pool(name="accs", bufs=2) as accp,
            tc.tile_pool(name="osb", bufs=2) as osp,
            tc.tile_pool(name="psproj", bufs=2, space="PSUM") as prp,
            tc.tile_pool(name="psst", bufs=2, space="PSUM") as stp,
            tc.tile_pool(name="pso", bufs=1, space="PSUM") as outp,
            tc.tile_pool(name="psl", bufs=1, space="PSUM") as lp,
        ):
            xt_view = xt_d.rearrange("(c p) s -> p c s", p=128)
            xT_tiles = {}

            def dma_xT(i2, pieces=2, first_single=False):
                xT = xtp.tile([128, 8, SUP], BF16, tag="xT", name="xT")
                if first_single:
                    # chunk 0 alone so the first Q-proj matmul starts ASAP
                    splits = [(0, 1), (1, 3), (4, 4)]
                else:
                    w = 8 // pieces
                    splits = [(w * hh, w) for hh in range(pieces)]
                for lo, w in splits:
                    nc.sync.dma_start(
                        xT[:, lo : lo + w, :],
                        xt_view[:, lo : lo + w, i2 * SUP : (i2 + 1) * SUP],
                    )
                xT_tiles[i2] = xT

            # DMA order tuned for PE start latency: wq chunk 0 + first x^T
            # chunk first, then the rest.
            w_r = {}
            wq_s = pp.tile([128, 8, H], BF16, name="wr_q")
            wq_view = wq_d.rearrange("(c p) h -> p c h", p=128)
            nc.sync.dma_start(wq_s[:, 0:1, :], wq_view[:, 0:1, :])
            w_r["q"] = wq_s
            dma_xT(0, first_single=True)
            nc.sync.dma_start(wq_s[:, 1:8, :], wq_view[:, 1:8, :])
            bias_q = pp.tile([128, 1], F32)
            nc.sync.dma_start(bias_q, bias_d)
            for nm, wd in (("k", wk_d), ("v", wv_d)):
                wr = pp.tile([128, 8, H], BF16, name=f"wr_{nm}")
                nc.sync.dma_start(wr, wd.rearrange("(c p) h -> p c h", p=128))
                w_r[nm] = wr
            dma_xT(1)

            ident = pp.tile([128, 128], BF16)
            make_identity(nc, ident)
            ones_col = pp.tile([128, 1], BF16)
            nc.vector.memset(ones_col, 1.0)

            mask_s = pp.tile([128, 2, SUP], BF16)
            nc.sync.dma_start(mask_s, mask_d)

            qt_all = pp.tile([128, S], BF16)
            kt_all = pp.tile([128, 16, 128], BF16)
            v_all = pp.tile([128, 16, 128], BF16)

            o_ps = {}
            l_ps = {}
            acc_of = {}
            pt_of = {}
            # per-R pair processing order: diagonal pair FIRST (it carries the
            # mask-mul, keeping DVE off the end-of-R critical path), then 0..R-1
            seq_of = {}
            for _R in range(NSUP):
                order = [_R] + list(range(_R))
                for _i, _p in enumerate(order):
                    seq_of[(_R, _p)] = (_i == 0, _i == len(order) - 1)

            def emit_A(i2):
                if i2 + 2 < NSUP:
                    dma_xT(i2 + 2)
                xT = xT_tiles.pop(i2)
                xT4 = xT.rearrange("p c (t w) -> p c t w", t=4)

                qt_ps = prp.tile([128, SUP], F32, tag="proj", name="qt_ps")
                for c in range(8):
                    nc.tensor.matmul(
                        qt_ps,
                        w_r["q"][:, c, :],
                        xT[:, c, :],
                        start=(c == 0),
                        stop=(c == 7),
                    )
                nc.scalar.activation(
                    qt_all[:, i2 * SUP : (i2 + 1) * SUP], qt_ps, ACT_IDENT, bias=bias_q
                )

                kt_ps = prp.tile([128, 256], F32, tag="proj", name="kt_ps")
                for c in range(8):
                    nc.tensor.matmul(
                        kt_ps,
                        w_r["k"][:, c, :],
                        xT4[:, c, 0::2, :],
                        start=(c == 0),
                        stop=(c == 7),
                    )
                nc.vector.tensor_copy(
                    kt_all[:, 2 * i2 : 2 * i2 + 2, :],
                    kt_ps.rearrange("p (t w) -> p t w", t=2),
                )

                vt_ps = prp.tile([128, 256], F32, tag="proj", name="vt_ps")
                for c in range(8):
                    nc.tensor.matmul(
                        vt_ps,
                        w_r["v"][:, c, :],
                        xT4[:, c, 0::2, :],
                        start=(c == 0),
                        stop=(c == 7),
                    )
                vt_tmp = vtp.tile([128, 256], BF16, tag="vtmp", name="vt_tmp")
                nc.vector.tensor_copy(vt_tmp, vt_ps)
                v_ps2 = prp.tile([128, 256], BF16, tag="proj", name="v_ps2")
                for tt in range(2):
                    nc.tensor.transpose(
                        v_ps2[:, tt * 128 : (tt + 1) * 128],
                        vt_tmp[:, tt * 128 : (tt + 1) * 128],
                        ident,
                    )
                nc.vector.tensor_copy(
                    v_all[:, 2 * i2 : 2 * i2 + 2, :],
                    v_ps2.rearrange("p (t w) -> p t w", t=2),
                )

            def emit_S(task):
                R, p = task
                first_p, _ = seq_of[task]
                st = stp.tile([128, 2, SUP], F32, tag="st", name="st")
                for half in range(2):
                    k = 2 * p + half
                    nc.tensor.matmul(
                        st[:, half, :],
                        kt_all[:, k, :],
                        qt_all[:, R * SUP : (R + 1) * SUP],
                        start=True,
                        stop=True,
                    )
                if first_p:
                    # diagonal pair: exp straight into the l-accumulator + mask
                    acc = accp.tile([128, 2, SUP], BF16, tag="acc", name="acc")
                    nc.scalar.activation(acc, st, ACT_EXP, scale=SCALE)
                    nc.vector.tensor_mul(acc, acc, mask_s)
                    acc_of[R] = acc
                    pt_of[task] = acc
                else:
                    pt = ptp.tile([128, 2, SUP], BF16, tag="pt", name="pt")
                    nc.scalar.activation(pt, st, ACT_EXP, scale=SCALE)
                    pt_of[task] = pt

            def emit_AV(task):
                R, p = task
                first_p, last_p = seq_of[task]
                if first_p:
                    o_ps[R] = outp.tile([128, SUP], F32, tag="o", name="o_ps")
                pt = pt_of.pop(task)
                if not first_p:
                    # accumulate into the l-sum tile; emitted here (one task
                    # after emit_S) so it orders AFTER emit_AV(diag)'s reads
                    # of acc — the diagonal pair's AV consumes acc as its P.
                    nc.vector.tensor_add(acc_of[R], acc_of[R], pt)
                for half in range(2):
                    k = 2 * p + half
                    nc.tensor.matmul(
                        o_ps[R],
                        v_all[:, k, :],
                        pt[:, half, :],
                        start=(first_p and half == 0),
                        stop=(last_p and half == 1),
                    )
                if last_p:
                    acc = acc_of.pop(R)
                    l_ps[R] = lp.tile([128, SUP], F32, tag="l", name="l_ps")
                    for half in range(2):
                        nc.tensor.matmul(
                            l_ps[R][0:1, :],
                            ones_col,
                            acc[:, half, :],
                            start=(half == 0),
                            stop=(half == 1),
                        )
                    o_sb = osp.tile([128, SUP], F32, tag="o_sb", name="o_sb")
                    nc.vector.tensor_copy(o_sb, o_ps[R])
                    nc.sync.dma_start(outT_d[:, R * SUP : (R + 1) * SUP], o_sb)
                    l_sb = osp.tile([1, SUP], F32, tag="l_sb", name="l_sb")
                    nc.vector.tensor_copy(l_sb, l_ps[R][0:1, :])
                    nc.sync.dma_start(l_d[:, R * SUP : (R + 1) * SUP], l_sb)

            pipe = {"prev": None}

            def push_task(task):
                emit_S(task)
                if pipe["prev"] is not None:
                    emit_AV(pipe["prev"])
                pipe["prev"] = task

            def emit_body():
                pipe["prev"] = None
                for i2 in range(NSUP):
                    emit_A(i2)
                    # superblock R=i2 is fully ready after A(i2): its qt slice
                    # is written there and its t-blocks k<=2*i2+1 live in
                    # supersteps k//2 <= i2. Diagonal pair first.
                    for p in [i2] + list(range(i2)):
                        push_task((i2, p))
                emit_AV(pipe["prev"])

            if loop_n is None:
                emit_body()
            else:
                with tc.For_i(0, loop_n, 1):
                    emit_body()

    nc.compile()
    return nc


def _perm1():
    idx = np.arange(S)
    return (idx // 128 ^ 1) * 128 + idx % 128


def _mask_for(j):
    ti = np.arange(128)[:, None, None]
    m = np.arange(2)[None, :, None]
    si = np.arange(SUP)[None, None, :]
    orig_s = 128 * ((si // 128) ^ j) + si % 128
    vis = orig_s >= 128 * (2 * m + j) + ti
    return np.where(vis, np.float32(1.0), np.float32(0.0)).astype(NP_BF16)


_CACHE = {}


def kernel(x, Wq, bq, Wk, bk, Wv, bv):
    if "nc" not in _CACHE:
        _CACHE["nc"] = build_nc()
    nc = _CACHE["nc"]

    x = np.asarray(x, dtype=np.float32)
    Wq = np.ascontiguousarray(np.asarray(Wq, dtype=np.float32)).astype(NP_BF16)
    Wk = np.ascontiguousarray(np.asarray(Wk, dtype=np.float32)).astype(NP_BF16)
    Wv = np.ascontiguousarray(np.asarray(Wv, dtype=np.float32)).astype(NP_BF16)
    bq = np.ascontiguousarray(np.asarray(bq, dtype=np.float32))
    bv = np.asarray(bv, dtype=np.float32)

    perm = _perm1()
    masks = {j: _mask_for(j) for j in (0, 1)}
    # x^T per batch (bf16), and the column-block-swapped variant for parity-1
    xT = {}
    for b in range(B):
        t = np.ascontiguousarray(x[b].T.astype(NP_BF16))  # [E, S]
        xT[(b, 0)] = t
        xT[(b, 1)] = np.ascontiguousarray(
            t.reshape(E, S // 128, 128)[:, (np.arange(S // 128) ^ 1), :].reshape(E, S)
        )

    in_maps = []
    for c in range(8):
        b, j = divmod(c, 2)
        in_maps.append(
            {
                "xt": xT[(b, j)],
                "wq": Wq,
                "wk": Wk,
                "wv": Wv,
                "bias": np.ascontiguousarray(bq.reshape(H, 1)),
                "mask": masks[j],
            }
        )

    res = bass_utils.run_bass_kernel_spmd(nc, in_maps, core_ids=list(range(8)))

    out = np.empty((B, S, H), np.float32)
    for b in range(B):
        oT0 = res.results[2 * b]["outT"]
        l0 = res.results[2 * b]["lsum"][0]
        oT1 = res.results[2 * b + 1]["outT"][:, perm]
        l1 = res.results[2 * b + 1]["lsum"][0][perm]
        out[b] = ((oT0 + oT1) / (l0 + l1)[None, :]).T + bv[None, :]
    return out


# revision 32
# speedup vs baseline: 1.2869x; 1.0995x over previous
"""Single-head causal self-attention on 8 trn2 NeuronCores.

Problem: x[4,4096,1024], Wq/Wk/Wv[1024,128]+biases -> causal attention out
[4,4096,128], fp32.

Sharding: core c = (b = c//2, j = c%2). Core (b, j) handles batch b and the
K/V column 128-blocks of parity j (alternating blocks balance the causal
triangle). It computes, for ALL 4096 query rows, the *unnormalized* partial
attention over its own columns:
    O_un^T[h, s] = sum_{t in cols_j, t<=s} exp(q_s.k_t * scale) * v_t[h]
    l[s]         = sum_{t in cols_j, t<=s} exp(q_s.k_t * scale)
Host combines:  O[s] = (O_un0[s] + O_un1[s]) / (l0[s] + l1[s]) + bv.
No per-core max subtraction is needed: scores are ~N(0,1) (bounded ~6), so
exp never overflows; masked entries get multiplied by a 0/1 mask after exp.

Bias algebra: softmax((q+bq).(k+bk)) == softmax((q+bq).k) because the
(q+bq).bk term is constant in t and cancels in the combined normalization
(both cores use the same biased-Q/unbiased-K convention). The V bias moves
to the host: O = sum w (v+bv) / sum w = sum w v / sum w + bv. So the device
applies only the Q bias.

SPMD uniformity: the same Bass program runs on all 8 cores. Parity enters
only through data: for j=1 the host swaps adjacent 128-row blocks of x
(involution), so "even position blocks" on the device are the core's own
columns; the causal masks (2 tiles of [128,512], R-independent by algebra)
are passed as per-core inputs. Output comes back in position space and the
host un-swaps.

Precision: all matmul operands are bf16 (x^T, W, Q^T, K^T, V, P); PSUM
accumulation is f32; exp input (scores) is f32. Measured end-to-end rel
err ~4e-3 vs the f32 reference (tolerance 2e-2).

Device pipeline per core (per 512-row superstep i2):
  Stage A: DMA x^T tile [128, 8, 512] bf16 -> projections with W stationary:
    Q^T (all s, ACT copy+bias -> bf16), K^T / V^T (even position blocks only,
    DVE copy -> bf16); V^T PE-transposed to V[t,h] (bf16 identity).
  Stage B (superblock R = i2, pairs of own t-blocks, DIAGONAL pair first):
    S^T[t,s] = K^T.T @ Q^T  (PSUM f32, 2 banks per pair)
    ACT: exp(scale * S^T) -> bf16. Diagonal pair writes straight into the
    l-accumulator tile and gets the 0/1 mask (DVE mul); later pairs write a
    pt tile and DVE-add into the accumulator.
    O^T += V.T @ P^T (PSUM accum over all 2R+2 blocks)
    After the last pair: l[1,512] = ones.T @ acc (2 matmuls, PSUM-accumulated
    over the two halves) -> copies -> DMA O^T[128,512] f32 and l[1,512] f32.
"""

import sys

sys.path.insert(0, "/opt/trn_rl_repo")

import numpy as np

import concourse.bacc as bacc
import concourse.mybir as mybir
import concourse.tile as tile
from concourse import bass_utils
from concourse.masks import make_identity

S, E, H, B = 4096, 1024, 128, 4
NSUP, SUP = 8, 512
SCALE = 1.0 / float(np.sqrt(128.0))
F32 = mybir.dt.float32
BF16 = mybir.dt.bfloat16
ACT_IDENT = mybir.ActivationFunctionType.Identity
ACT_EXP = mybir.ActivationFunctionType.Exp

try:
    import ml_dtypes

    NP_BF16 = ml_dtypes.bfloat16
except ImportError:  # pragma: no cover
    NP_BF16 = None


def build_nc(loop_n=None, warmup=14):
    nc = bacc.Bacc("TRN2", debug=False, num_devices=8)
    xt_d = nc.dram_tensor("xt", [E, S], BF16, kind="ExternalInput").ap()
    # weights arrive host-pre-arranged as [p, c*H] so DMA descriptors are
    # 2KB/partition (full line rate) instead of 256B (2x latency penalty)
    wq_d = nc.dram_tensor("wq", [128, 8 * H], BF16, kind="ExternalInput").ap()
    wk_d = nc.dram_tensor("wk", [128, 8 * H], BF16, kind="ExternalInput").ap()
    wv_d = nc.dram_tensor("wv", [128, 8 * H], BF16, kind="ExternalInput").ap()
    bias_d = nc.dram_tensor("bias", [H, 1], F32, kind="ExternalInput").ap()
    # trimmed diagonal mask: cols [0:512] = first block over all s, cols
    # [512:768] = second block over s in [256:512) (no unmasked content below)
    mask_d = nc.dram_tensor("mask", [128, 768], BF16, kind="ExternalInput").ap()
    outT_d = nc.dram_tensor("outT", [H, S], BF16, kind="ExternalOutput").ap()
    l_d = nc.dram_tensor("lsum", [1, S], F32, kind="ExternalOutput").ap()

    with tile.TileContext(nc) as tc:
        with (
            tc.tile_pool(name="persist", bufs=1) as pp,
            tc.tile_pool(name="xts", bufs=3) as xtp,
            tc.tile_pool(name="vtmp", bufs=2) as vtp,
            tc.tile_pool(name="pts", bufs=4) as ptp,
            tc.tile_pool(name="accs", bufs=2) as accp,
            tc.tile_pool(name="osb", bufs=2) as osp,
            tc.tile_pool(name="psproj", bufs=2, space="PSUM") as prp,
            tc.tile_pool(name="psst", bufs=2, space="PSUM") as stp,
            tc.tile_pool(name="pso", bufs=1, space="PSUM") as outp,
            tc.tile_pool(name="psl", bufs=1, space="PSUM") as lp,
        ):
            xt_view = xt_d.rearrange("(c p) s -> p c s", p=128)
            xT_tiles = {}

            def dma_xT_piece(i2, lo, w):
                if i2 not in xT_tiles:
                    xT_tiles[i2] = xtp.tile([128, 8, SUP], BF16, tag="xT", name="xT")
                nc.sync.dma_start(
                    xT_tiles[i2][:, lo : lo + w, :],
                    xt_view[:, lo : lo + w, i2 * SUP : (i2 + 1) * SUP],
                )

            def dma_xT(i2):
                dma_xT_piece(i2, 0, 4)
                dma_xT_piece(i2, 4, 4)

            # Head DMA order tuned so S(0,0)'s gating inputs (wq, bias, xT0,
            # wk) transfer first on the serialized DMA bus; wv/mask can land
            # ~1.5us later (AV(0,0) waits on exp anyway); xT1 right after.
            w_r = {}
            wq_s = pp.tile([128, 8, H], BF16, name="wr_q")
            wq_view = wq_d.rearrange("p (c h) -> p c h", c=8)
            nc.sync.dma_start(wq_s[:, 0:1, :], wq_view[:, 0:1, :])
            w_r["q"] = wq_s
            bias_q = pp.tile([128, 1], F32)
            nc.sync.dma_start(bias_q, bias_d)
            dma_xT_piece(0, 0, 4)
            nc.sync.dma_start(wq_s[:, 1:8, :], wq_view[:, 1:8, :])
            dma_xT_piece(0, 4, 4)
            wr_k = pp.tile([128, 8, H], BF16, name="wr_k")
            nc.sync.dma_start(wr_k, wk_d.rearrange("p (c h) -> p c h", c=8))
            w_r["k"] = wr_k
            dma_xT_piece(1, 0, 4)
            wr_v = pp.tile([128, 8, H], BF16, name="wr_v")
            nc.sync.dma_start(wr_v, wv_d.rearrange("p (c h) -> p c h", c=8))
            w_r["v"] = wr_v
            dma_xT_piece(1, 4, 4)
            mask_s = pp.tile([128, 768], BF16)
            nc.sync.dma_start(mask_s, mask_d)

            ident = pp.tile([128, 128], BF16)
            make_identity(nc, ident)
            ones_col = pp.tile([128, 1], BF16)
            nc.vector.memset(ones_col, 1.0)

            # PE warm-up: dummy matmuls with no DMA deps bridge the head DMA
            # latency so the pstate ramp completes before (and PE never idles
            # ahead of) the first projection matmul. Source tile comes from a
            # fast DVE memset so warmups start within ~0.3us.
            if warmup:
                warm_src = pp.tile([128, 128], BF16, name="warm_src")
                nc.vector.memset(warm_src, 0.0)
                warm_ps = prp.tile([128, 128], F32, tag="proj", name="warm_ps")
                for _ in range(warmup):
                    nc.tensor.matmul(warm_ps, warm_src, warm_src, start=True, stop=True)

            qt_all = pp.tile([128, S], BF16)
            kt_all = pp.tile([128, 16, 128], BF16)
            v_all = pp.tile([128, 16, 128], BF16)

            o_ps = {}
            l_ps = {}
            acc_of = {}
            pt_of = {}
            # emission-order first/last flags per (R, p), filled by emit_body
            seq_of = {}

            def emit_A_q(i2):
                if i2 + 2 < NSUP:
                    dma_xT(i2 + 2)
                xT = xT_tiles[i2]
                qt_ps = prp.tile([128, SUP], F32, tag="proj", name="qt_ps")
                for c in range(8):
                    nc.tensor.matmul(
                        qt_ps,
                        w_r["q"][:, c, :],
                        xT[:, c, :],
                        start=(c == 0),
                        stop=(c == 7),
                    )
                nc.vector.tensor_scalar_add(
                    qt_all[:, i2 * SUP : (i2 + 1) * SUP], qt_ps, bias_q
                )

            def emit_A_k(i2):
                xT = xT_tiles[i2]
                xT4 = xT.rearrange("p c (t w) -> p c t w", t=4)
                kt_ps = prp.tile([128, 256], F32, tag="proj", name="kt_ps")
                for c in range(8):
                    nc.tensor.matmul(
                        kt_ps,
                        w_r["k"][:, c, :],
                        xT4[:, c, 0::2, :],
                        start=(c == 0),
                        stop=(c == 7),
                    )
                nc.vector.tensor_copy(
                    kt_all[:, 2 * i2 : 2 * i2 + 2, :],
                    kt_ps.rearrange("p (t w) -> p t w", t=2),
                )

            def emit_A_v(i2):
                xT = xT_tiles.pop(i2)
                xT4 = xT.rearrange("p c (t w) -> p c t w", t=4)
                vt_ps = prp.tile([128, 256], F32, tag="proj", name="vt_ps")
                for c in range(8):
                    nc.tensor.matmul(
                        vt_ps,
                        w_r["v"][:, c, :],
                        xT4[:, c, 0::2, :],
                        start=(c == 0),
                        stop=(c == 7),
                    )
                vt_tmp = vtp.tile([128, 256], BF16, tag="vtmp", name="vt_tmp")
                nc.scalar.activation(vt_tmp, vt_ps, ACT_IDENT)
                v_ps2 = prp.tile([128, 256], BF16, tag="proj", name="v_ps2")
                for tt in range(2):
                    nc.tensor.transpose(
                        v_ps2[:, tt * 128 : (tt + 1) * 128],
                        vt_tmp[:, tt * 128 : (tt + 1) * 128],
                        ident,
                    )
                nc.vector.tensor_copy(
                    v_all[:, 2 * i2 : 2 * i2 + 2, :],
                    v_ps2.rearrange("p (t w) -> p t w", t=2),
                )

            def emit_S(task):
                R, p = task
                first_p, _ = seq_of[task]
                qt_R = qt_all[:, R * SUP : (R + 1) * SUP]
                if p == R:
                    # diagonal pair, trimmed: block 2R over all 512 s-cols,
                    # block 2R+1 only over s in [256:512) (rest is masked)
                    st = stp.tile([128, 768], F32, tag="st", name="st_d")
                    nc.tensor.matmul(
                        st[:, 0:SUP], kt_all[:, 2 * R, :], qt_R, start=True, stop=True
                    )
                    nc.tensor.matmul(
                        st[:, SUP : SUP + 256],
                        kt_all[:, 2 * R + 1, :],
                        qt_all[:, R * SUP + 256 : (R + 1) * SUP],
                        start=True,
                        stop=True,
                    )
                    if first_p:
                        pt = accp.tile([128, 768], BF16, tag="acc", name="acc_d")
                        acc_of[R] = pt
                    else:
                        pt = ptp.tile([128, 768], BF16, tag="pt", name="pt_d")
                    nc.scalar.activation(pt, st, ACT_EXP, scale=SCALE)
                    nc.vector.tensor_mul(pt, pt, mask_s)
                    pt_of[task] = pt
                else:
                    st = stp.tile([128, 2, SUP], F32, tag="st", name="st")
                    for half in range(2):
                        k = 2 * p + half
                        nc.tensor.matmul(
                            st[:, half, :], kt_all[:, k, :], qt_R, start=True, stop=True
                        )
                    if first_p:
                        # first-emitted pair's exp lands straight in the l-acc
                        pt = accp.tile([128, 2, SUP], BF16, tag="acc", name="acc")
                        acc_of[R] = pt
                    else:
                        pt = ptp.tile([128, 2, SUP], BF16, tag="pt", name="pt")
                    nc.scalar.activation(pt, st, ACT_EXP, scale=SCALE)
                    pt_of[task] = pt

            def emit_AV(task):
                R, p = task
                first_p, last_p = seq_of[task]
                if first_p:
                    o_ps[R] = outp.tile([128, SUP], F32, tag="o", name="o_ps")
                pt = pt_of.pop(task)
                diag = p == R
                # skip the final DVE accumulate on the very last pair of the
                # last superblock: its contribution is streamed straight into
                # the l matmuls instead (shorter end-of-kernel chain)
                skip_add = last_p and R == NSUP - 1 and not first_p
                if not first_p and not skip_add:
                    # accumulate into the l-sum tile; emitted here (one task
                    # after emit_S) so it orders AFTER emit_AV(first)'s reads
                    # of acc — the first pair's AV consumes acc as its P.
                    acc = acc_of[R]
                    if diag:
                        accf = acc.rearrange("p a b -> p (a b)")
                        nc.vector.tensor_add(accf[:, 0:SUP], accf[:, 0:SUP], pt[:, 0:SUP])
                        nc.vector.tensor_add(
                            accf[:, SUP + 256 : 2 * SUP],
                            accf[:, SUP + 256 : 2 * SUP],
                            pt[:, SUP : SUP + 256],
                        )
                    else:
                        nc.vector.tensor_add(acc, acc, pt)
                if diag:
                    nc.tensor.matmul(
                        o_ps[R],
                        v_all[:, 2 * R, :],
                        pt[:, 0:SUP],
                        start=first_p,
                        stop=False,
                    )
                    nc.tensor.matmul(
                        o_ps[R][:, 256:SUP],
                        v_all[:, 2 * R + 1, :],
                        pt[:, SUP : SUP + 256],
                        start=False,
                        stop=last_p,
                    )
                else:
                    for half in range(2):
                        k = 2 * p + half
                        nc.tensor.matmul(
                            o_ps[R],
                            v_all[:, k, :],
                            pt[:, half, :],
                            start=(first_p and half == 0),
                            stop=(last_p and half == 1),
                        )
                if last_p:
                    acc = acc_of.pop(R)
                    l_ps[R] = lp.tile([128, SUP], F32, tag="l", name="l_ps")
                    if R == 0:
                        # acc is the trimmed diagonal tile [128, 768]
                        nc.tensor.matmul(
                            l_ps[R][0:1, :], ones_col, acc[:, 0:SUP], start=True, stop=False
                        )
                        nc.tensor.matmul(
                            l_ps[R][0:1, 256:SUP],
                            ones_col,
                            acc[:, SUP : SUP + 256],
                            start=False,
                            stop=True,
                        )
                    else:
                        l_srcs = [acc[:, 0, :], acc[:, 1, :]]
                        if skip_add:
                            l_srcs += [pt[:, 0, :], pt[:, 1, :]]
                        for i, src in enumerate(l_srcs):
                            nc.tensor.matmul(
                                l_ps[R][0:1, :],
                                ones_col,
                                src,
                                start=(i == 0),
                                stop=(i == len(l_srcs) - 1),
                            )
                    o_sb = osp.tile([128, SUP], BF16, tag="o_sb", name="o_sb")
                    # ACT paces the last superblocks (8 exps each); route the
                    # late o drains to DVE which has slack there
                    if R < 5:
                        nc.scalar.activation(o_sb, o_ps[R], ACT_IDENT)
                    else:
                        nc.vector.tensor_copy(o_sb, o_ps[R])
                    nc.sync.dma_start(outT_d[:, R * SUP : (R + 1) * SUP], o_sb)
                    l_sb = osp.tile([1, SUP], F32, tag="l_sb", name="l_sb")
                    nc.vector.tensor_copy(l_sb, l_ps[R][0:1, :])
                    nc.sync.dma_start(l_d[:, R * SUP : (R + 1) * SUP], l_sb)

            pipe = {"prev": None}

            def push_task(task):
                emit_S(task)
                if pipe["prev"] is not None:
                    emit_AV(pipe["prev"])
                pipe["prev"] = task

            def emit_body():
                pipe["prev"] = None
                for i2 in range(NSUP):
                    # non-diagonal pairs p<=i2-2 need only qt(i2) and OLD
                    # kt/v blocks, so they interleave with this superstep's
                    # K/V projections; the diagonal pair (needs fresh kt)
                    # goes right after A_k, and pair i2-1 last to keep the
                    # 1-deep S->AV pipeline around the diagonal.
                    # pair i2-2 sits between A_k and A_v so the fresh kt
                    # DVE-copy latency hides under its S/AV work; the diagonal
                    # goes after A_v (its S covers the vt-copy->transpose
                    # latency) except at i2=0 where A_v's wv DMA arrives late.
                    if i2 == 0:
                        before, after_k, after_v = [], [0], []
                    elif i2 == 1:
                        before, after_k, after_v = [0], [], [1]
                    else:
                        before = list(range(i2 - 2))
                        after_k = [i2 - 2]
                        after_v = [i2, i2 - 1]
                    order = before + after_k + after_v
                    for _i, _p in enumerate(order):
                        seq_of[(i2, _p)] = (_i == 0, _i == len(order) - 1)
                    emit_A_q(i2)
                    for p in before:
                        push_task((i2, p))
                    emit_A_k(i2)
                    for p in after_k:
                        push_task((i2, p))
                    emit_A_v(i2)
                    for p in after_v:
                        push_task((i2, p))
                emit_AV(pipe["prev"])

            if loop_n is None:
                emit_body()
            else:
                with tc.For_i(0, loop_n, 1):
                    emit_body()

    nc.compile()
    return nc


def _perm1():
    idx = np.arange(S)
    return (idx // 128 ^ 1) * 128 + idx % 128


def _mask_for(j):
    ti = np.arange(128)[:, None, None]
    m = np.arange(2)[None, :, None]
    si = np.arange(SUP)[None, None, :]
    orig_s = 128 * ((si // 128) ^ j) + si % 128
    vis = (orig_s >= 128 * (2 * m + j) + ti).astype(np.float32)
    # trimmed layout [128, 768]: block 0 over all 512 s, block 1 only over
    # s in [256:512) (the rest of block 1 has no unmasked content)
    return np.concatenate([vis[:, 0, :], vis[:, 1, 256:]], axis=1).astype(NP_BF16)


_CACHE = {}


def kernel(x, Wq, bq, Wk, bk, Wv, bv):
    if "nc" not in _CACHE:
        _CACHE["nc"] = build_nc()
    nc = _CACHE["nc"]

    def _w_arrange(W):
        # [E, H] -> [128, 8*H] with [p, c*H+h] = W[c*128+p, h]
        W = np.asarray(W, dtype=np.float32).astype(NP_BF16)
        return np.ascontiguousarray(
            W.reshape(8, 128, H).transpose(1, 0, 2).reshape(128, 8 * H)
        )

    x = np.asarray(x, dtype=np.float32)
    Wq = _w_arrange(Wq)
    Wk = _w_arrange(Wk)
    Wv = _w_arrange(Wv)
    bq = np.ascontiguousarray(np.asarray(bq, dtype=np.float32))
    bv = np.asarray(bv, dtype=np.float32)

    perm = _perm1()
    masks = {j: _mask_for(j) for j in (0, 1)}
    # x^T per batch (bf16), and the column-block-swapped variant for parity-1
    xT = {}
    for b in range(B):
        t = np.ascontiguousarray(x[b].T.astype(NP_BF16))  # [E, S]
        xT[(b, 0)] = t
        xT[(b, 1)] = np.ascontiguousarray(
            t.reshape(E, S // 128, 128)[:, (np.arange(S // 128) ^ 1), :].reshape(E, S)
        )

    in_maps = []
    for c in range(8):
        b, j = divmod(c, 2)
        in_maps.append(
            {
                "xt": xT[(b, j)],
                "wq": Wq,
                "wk": Wk,
                "wv": Wv,
                "bias": np.ascontiguousarray(bq.reshape(H, 1)),
                "mask": masks[j],
            }
        )

    res = bass_utils.run_bass_kernel_spmd(nc, in_maps, core_ids=list(range(8)))

    out = np.empty((B, S, H), np.float32)
    for b in range(B):
        oT0 = res.results[2 * b]["outT"].astype(np.float32)
        l0 = res.results[2 * b]["lsum"][0]
        oT1 = res.results[2 * b + 1]["outT"].astype(np.float32)[:, perm]
        l1 = res.results[2 * b + 1]["lsum"][0][perm]
        out[b] = ((oT0 + oT1) / (l0 + l1)[None, :]).T + bv[None, :]
    return out
